# revision 1
# baseline (speedup 1.0000x reference)
"""Bahdanau-style additive attention on 8 TRN2 NeuronCores.

score(n, l) = v . tanh(decoder_hidden[n] @ W_h.T + encoder_hiddens[n, l] @ W_s.T)
attn = softmax(mask(score));  context[n] = attn[n] @ encoder_hiddens[n]

Sharding: data-parallel over batch N=64 -> 8 batches per core, weights
replicated, no collectives.

Device layouts (prepared host-side so the device never transposes the big
encoder tensor):
  eT   [8, H, L] bf16  - per-batch transposed encoder (for the W_s matmul,
                         contraction over h must sit on SBUF partitions)
  eN   [8, L, H] bf16  - natural layout (for the context matmul, contraction
                         over l on partitions)
  wsT  [H, H]    bf16  - W_s.T            whT [H, H] bf16 - W_h.T
  decT [H, 8]    bf16  - decoder shard transposed
  vcol [128, 8]  bf16  - v reshaped so chunk c lives at [:, c]
  mneg [8, L]    f32   - -1e30 where PAD else 0
Compute: bf16 matmuls with f32 PSUM accumulation, f32 softmax.
"""

import os
import numpy as np
import ml_dtypes

N_CORES = 8
N, L, H = 64, 1024, 1024
NB = N // N_CORES  # batches per core
P = 128
HC = H // P  # h chunks
LC = L // P  # l chunks
KC = H // P  # k (output-dim) chunks

_cache = {}

last_exec_time_ns = None
last_trace = None


def _build():
    import concourse.bass as bass
    import concourse.bacc as bacc
    import concourse.tile as tile
    from concourse import mybir

    f32 = mybir.dt.float32
    bf16 = mybir.dt.bfloat16
    TANH = mybir.ActivationFunctionType.Tanh
    EXP = mybir.ActivationFunctionType.Exp

    nc = bacc.Bacc("TRN2", target_bir_lowering=False, debug=False,
                   num_devices=N_CORES)

    eT = nc.dram_tensor("eT", [NB, H, L], bf16, kind="ExternalInput")
    eN = nc.dram_tensor("eN", [NB, L, H], bf16, kind="ExternalInput")
    wsT = nc.dram_tensor("wsT", [H, H], bf16, kind="ExternalInput")
    whT = nc.dram_tensor("whT", [H, H], bf16, kind="ExternalInput")
    decT = nc.dram_tensor("decT", [P, HC * NB], bf16, kind="ExternalInput")
    vcol = nc.dram_tensor("vcol", [P, HC], bf16, kind="ExternalInput")
    mneg = nc.dram_tensor("mneg", [NB, P, L // 4], f32, kind="ExternalInput")
    ctx_out = nc.dram_tensor("ctx", [NB, H], f32, kind="ExternalOutput")
    attn_out = nc.dram_tensor("attn", [NB, L], f32, kind="ExternalOutput")

    with tile.TileContext(nc) as tc:
        with (
            tc.tile_pool(name="const", bufs=1) as cpool,
            tc.tile_pool(name="et", bufs=2) as etpool,
            tc.tile_pool(name="en", bufs=2) as enpool,
            tc.tile_pool(name="work", bufs=3) as wpool,
            tc.tile_pool(name="rows", bufs=2) as rpool,
            tc.tile_pool(name="ps", bufs=2, space=bass.MemorySpace.PSUM) as ppool,
            tc.tile_pool(name="ps1", bufs=1, space=bass.MemorySpace.PSUM) as ppool1,
            tc.tile_pool(name="psrow", bufs=2, space=bass.MemorySpace.PSUM) as prow,
        ):
            # ---- load replicated weights; order shapes DMA queue order:
            # dec+wh first (feeds dh matmuls), then ws / et(n=0)
            # interleaved so the main matmul stream can start early ----
            ws_sb = cpool.tile([P, HC, H], bf16)   # [p, hc, k] = wsT[hc*P+p, k]
            wh_sb = cpool.tile([P, HC, H], bf16)
            dec_sb = cpool.tile([P, HC, NB], bf16)
            nc.sync.dma_start(dec_sb[:], decT[:, :])
            for hc in range(HC):
                nc.sync.dma_start(wh_sb[:, hc, :], whT[hc * P:(hc + 1) * P, :])
            v_sb = cpool.tile([P, HC], bf16)
            nc.sync.dma_start(v_sb[:], vcol[:, :])

            et0_sb = etpool.tile([P, HC, L], bf16, tag="et")
            for hc in range(HC):
                nc.sync.dma_start(ws_sb[:, hc, :], wsT[hc * P:(hc + 1) * P, :])
                nc.sync.dma_start(et0_sb[:, hc, :], eT[0, hc * P:(hc + 1) * P, :])

            # ---- PE warmup: ~4us of dense dummy matmuls so the HAM
            # clock gate reaches 8/8 before the real stream starts ----
            warm_sb = cpool.tile([P, P], bf16)
            nc.vector.memset(warm_sb[:], 0.0)
            warm_ps = ppool1.tile([P, P], f32, tag="pc")
            for i in range(190):
                nc.tensor.matmul(warm_ps[:], warm_sb[:], warm_sb[:],
                                 start=True, stop=True)

            # ---- dh^T[k, n] = sum_h W_h[k, h] * dec[n, h] ----
            dhT_sb = cpool.tile([P, KC, NB], f32)

            def emit_dh():
                for kc in range(KC):
                    ps = ppool.tile([P, NB], f32, tag="ehps")
                    for hc in range(HC):
                        nc.tensor.matmul(
                            ps[:],
                            wh_sb[:, hc, kc * P:(kc + 1) * P],
                            dec_sb[:, hc, :],
                            start=(hc == 0), stop=(hc == HC - 1))
                    nc.vector.tensor_copy(dhT_sb[:, kc, :], ps[:])

            # ---- fused per-batch pipeline ----
            # ones on every partition (outer-product rhs for any row base)
            ones_sb = cpool.tile([P, 1], bf16)
            nc.vector.memset(ones_sb[:], 1.0)
            # Z-broadcast selector: ones at partitions {0,32,64,96} -> matmul
            # broadcasts the sum of the 4 per-quarter softmax sums to all
            # 128 output partitions
            selbc_sb = cpool.tile([P, P], f32)
            nc.vector.memset(selbc_sb[:], 0.0)
            for j in range(4):
                nc.vector.memset(selbc_sb[32 * j:32 * j + 1, :], 1.0)
            # per-batch mask tiles in split-row layout
            mneg_sb_all = cpool.tile([P, NB, L // 4], f32)
            for n in range(NB):
                nc.sync.dma_start(mneg_sb_all[:, n, :], mneg[n, :, :])
            # scrub the score PSUM slot once: quarters only ever write their
            # 4 rows; stale bits elsewhere must not be NaN/huge (exp reads
            # the full tile)
            sc_init_a = prow.tile([P, L // 4], f32, tag="row")
            nc.vector.memset(sc_init_a[:], 0.0)
            sc_init_b = prow.tile([P, L // 4], f32, tag="row")
            nc.vector.memset(sc_init_b[:], 0.0)
            QL = L // 4  # 256; quarter j lives at psum row 32j, cols 0:QL

            def sc_quads(sc_ps, n, kc, th):
                # score quarters: column group j -> psum row 32j, cols 0:QL
                for j in range(4):
                    nc.tensor.matmul(
                        sc_ps[32 * j:32 * j + 1, :],
                        v_sb[:, kc:kc + 1],
                        th[:, j * QL:(j + 1) * QL],
                        start=(kc == 0), stop=(kc == KC - 1),
                        tile_position=(0, 32 * j))

            def emit_tail(n, sc_ps, en_sb):
                # masked softmax, no max-subtraction (|score| <= sum|v| ~ 26,
                # exp stays in f32 range; mask adds -1e30 pre-exp)
                sc_m = rpool.tile([P, QL], f32, tag="scrow")
                nc.vector.tensor_add(sc_m[:], sc_ps[:], mneg_sb_all[:, n, :])
                prob = rpool.tile([P, QL], f32, tag="prob")
                zs4 = wpool.tile([P, 1], f32, tag="z4")
                nc.scalar.activation(prob[:], sc_m[:], EXP, accum_out=zs4[:])
                z_ps = ppool1.tile([P, 1], f32, tag="pc")
                nc.tensor.matmul(z_ps[:], selbc_sb[:], zs4[:],
                                 start=True, stop=True)
                rzb = wpool.tile([P, 1], f32, tag="rz")
                nc.vector.reciprocal(rzb[:], z_ps[:])
                arow_b = wpool.tile([P, QL], bf16, tag="arowb")
                nc.vector.tensor_scalar_mul(arow_b[:], prob[:], rzb[:])
                arow_f = rpool.tile([P, QL], f32, tag="arowf")
                nc.vector.tensor_scalar_mul(arow_f[:], prob[:], rzb[:])
                for j in range(4):
                    nc.sync.dma_start(
                        attn_out[n:n + 1, j * QL:(j + 1) * QL],
                        arow_f[32 * j:32 * j + 1, :])

                # transpose attn quarters -> columns via outer products
                ac_ps = ppool1.tile([P, LC], f32, tag="pc")
                for lc in range(LC):
                    j = lc // 2
                    nc.tensor.matmul(ac_ps[:, lc:lc + 1],
                                     arow_b[32 * j:32 * j + 1,
                                            (lc % 2) * P:(lc % 2 + 1) * P],
                                     ones_sb[32 * j:32 * j + 1, :],
                                     start=True, stop=True,
                                     tile_position=(32 * j, 0))
                acol = wpool.tile([P, LC], bf16, tag="acol")
                nc.vector.tensor_copy(acol[:], ac_ps[:])

                # context[n, h] = sum_l attn[l] E[l, h]; 4 column groups
                # compute disjoint h-quarters at psum rows 32j, cols 0:QH
                QH = H // 4
                cx_ps = ppool1.tile([P, QH], f32, tag="pc")
                for lc in range(LC):
                    for j in range(4):
                        nc.tensor.matmul(
                            cx_ps[32 * j:32 * j + 1, :],
                            acol[:, lc:lc + 1],
                            en_sb[:, lc, j * QH:(j + 1) * QH],
                            start=(lc == 0), stop=(lc == LC - 1),
                            tile_position=(0, 32 * j))
                cx_row = rpool.tile([P, QH], f32, tag="cxrow")
                nc.vector.tensor_copy(cx_row[:], cx_ps[:])
                for j in range(4):
                    nc.sync.dma_start(ctx_out[n:n + 1, j * QH:(j + 1) * QH],
                                      cx_row[32 * j:32 * j + 1, :])

            pend = None
            for n in range(NB):
                if n == 0:
                    et_sb = et0_sb
                else:
                    et_sb = etpool.tile([P, HC, L], bf16, tag="et")
                    for hc in range(HC):
                        nc.sync.dma_start(et_sb[:, hc, :],
                                          eT[n, hc * P:(hc + 1) * P, :])
                en_sb = enpool.tile([P, LC, H], bf16, tag="en")
                for lc in range(LC):
                    nc.sync.dma_start(en_sb[:, lc, :],
                                      eN[n, lc * P:(lc + 1) * P, :])

                sc_ps = prow.tile([P, QL], f32, tag="row")
                prev_th = None
                for kc in range(KC):
                    eh_ps = ppool.tile([P, L], f32, tag="ehps")
                    for hc in range(HC):
                        for lt in range(2):
                            nc.tensor.matmul(
                                eh_ps[:, lt * 512:(lt + 1) * 512],
                                ws_sb[:, hc, kc * P:(kc + 1) * P],
                                et_sb[:, hc, lt * 512:(lt + 1) * 512],
                                start=(hc == 0), stop=(hc == HC - 1))
                    if n == 0 and kc == 0:
                        emit_dh()
                    th = wpool.tile([P, L], bf16, tag="tanh")
                    nc.scalar.activation(th[:], eh_ps[:], TANH,
                                         bias=dhT_sb[:, kc, n:n + 1])
                    if kc == 2 and pend is not None:
                        # previous batch's softmax/attn/context, emitted two
                        # eh groups into this batch so the PE never waits
                        emit_tail(*pend)
                        pend = None
                    if prev_th is not None:
                        sc_quads(sc_ps, n, kc - 1, prev_th)
                    prev_th = th
                sc_quads(sc_ps, n, KC - 1, prev_th)
                pend = (n, sc_ps, en_sb)
            emit_tail(*pend)

    nc.compile()
    return nc


def kernel(decoder_hidden, encoder_hiddens, mask, W_h, W_s, v):
    global last_exec_time_ns, last_trace
    from concourse.bass_utils import run_bass_kernel_spmd

    bf16 = ml_dtypes.bfloat16
    dec = np.asarray(decoder_hidden, np.float32)
    enc = np.asarray(encoder_hiddens, np.float32)
    msk = np.asarray(mask)
    W_h = np.asarray(W_h, np.float32)
    W_s = np.asarray(W_s, np.float32)
    v = np.asarray(v, np.float32)

    wsT = np.ascontiguousarray(W_s.T).astype(bf16)
    whT = np.ascontiguousarray(W_h.T).astype(bf16)
    vcol = np.ascontiguousarray(v.reshape(HC, P).T).astype(bf16)
    NEG = np.float32(-1e30)
    mneg_rows = np.where(msk, NEG, np.float32(0.0)).astype(np.float32)  # [N, L]
    QL = L // 4
    mneg4 = np.full((N, P, QL), NEG, np.float32)
    for j in range(4):
        mneg4[:, 32 * j, :] = mneg_rows[:, j * QL:(j + 1) * QL]

    enc_b = enc.astype(bf16)

    in_maps = []
    for c in range(N_CORES):
        s = slice(c * NB, (c + 1) * NB)
        in_maps.append({
            "eT": np.ascontiguousarray(enc_b[s].transpose(0, 2, 1)),
            "eN": np.ascontiguousarray(enc_b[s]),
            "wsT": wsT,
            "whT": whT,
            "decT": np.ascontiguousarray(
                dec[s].T.reshape(HC, P, NB).transpose(1, 0, 2).reshape(P, HC * NB)
            ).astype(bf16),
            "vcol": vcol,
            "mneg": np.ascontiguousarray(mneg4[s]),
        })

    if "nc" not in _cache:
        _cache["nc"] = _build()
    nc = _cache["nc"]

    trace = bool(int(os.environ.get("BASS_KERNEL_TRACE", "0")))
    res = run_bass_kernel_spmd(nc, in_maps, core_ids=list(range(N_CORES)),
                               trace=trace)
    last_exec_time_ns = res.exec_time_ns
    last_trace = res.instructions_and_trace

    context = np.concatenate([res.results[c]["ctx"] for c in range(N_CORES)], 0)
    attn_w = np.concatenate([res.results[c]["attn"] for c in range(N_CORES)], 0)
    return (context.astype(np.float32), attn_w.astype(np.float32))



# revision 2
# speedup vs baseline: 1.2164x; 1.2164x over previous
"""Bahdanau-style additive attention on 8 TRN2 NeuronCores.

score(n, l) = v . tanh(decoder_hidden[n] @ W_h.T + encoder_hiddens[n, l] @ W_s.T)
attn = softmax(mask(score));  context[n] = attn[n] @ encoder_hiddens[n]

Sharding: data-parallel over batch N=64 -> 8 batches per core, weights
replicated, no collectives.

Mixed precision: the k (score-feature) dimension is permuted host-side so
that k-chunks are sorted by |v_k|. Score error sensitivity to eh noise
scales with |v_k|, so the low-|v| 5/8 of chunks run the W_s matmul in
fp8-e4m3 with DoubleRow (2x PE throughput); the high-|v| 3/8 stay bf16.
W_s is pre-scaled x16 for e4m3 (undone via the tanh activation's scale).

Device layouts (prepared host-side so the device never transposes the big
encoder tensor):
  eT   [8, H, L] bf16  - per-batch transposed encoder (bf16 k-chunks)
  eT8  [8, H, L] f8e4  - same, quantized e4m3 (fp8 k-chunks)
  eN   [8, L, H] bf16  - natural layout (context matmul, contraction over l)
  wsB  [H, KB*128] bf16 - W_s.T columns for the bf16 (high-|v|) chunks
  ws8  [H, KF*128] f8e4 - 16*W_s.T columns for the fp8 (low-|v|) chunks
  whT  [H, H] bf16     - W_h.T (k columns permuted)
  decT [128, HC*8] bf16 - decoder shard transposed
  vcol [128, 8]  bf16  - permuted v reshaped so chunk c lives at [:, c]
  mneg [8, L]    f32   - -1e30 where PAD else 0
Compute: bf16/fp8 matmuls with f32 PSUM accumulation, f32 softmax.
"""

import os
import numpy as np
import ml_dtypes

N_CORES = 8
N, L, H = 64, 1024, 1024
NB = N // N_CORES  # batches per core
P = 128
HC = H // P  # h chunks
LC = L // P  # l chunks
KC = H // P  # k (output-dim) chunks
KF = 5       # low-|v| k-chunks computed in fp8 DoubleRow
KB = KC - KF # high-|v| k-chunks computed in bf16
WS_SCALE = 16.0  # fp8 weight pre-scale (undone in tanh activation)

_cache = {}

last_exec_time_ns = None
last_trace = None


def _build():
    import concourse.bass as bass
    import concourse.bacc as bacc
    import concourse.tile as tile
    from concourse import mybir

    f32 = mybir.dt.float32
    bf16 = mybir.dt.bfloat16
    f8e4 = mybir.dt.float8e4
    TANH = mybir.ActivationFunctionType.Tanh
    EXP = mybir.ActivationFunctionType.Exp
    DROW = mybir.MatmulPerfMode.DoubleRow

    nc = bacc.Bacc("TRN2", target_bir_lowering=False, debug=False,
                   num_devices=N_CORES)

    eT = nc.dram_tensor("eT", [NB, H, L], bf16, kind="ExternalInput")
    eT8 = nc.dram_tensor("eT8", [NB, H, L], f8e4, kind="ExternalInput")
    eN = nc.dram_tensor("eN", [NB, L, H], bf16, kind="ExternalInput")
    wsB = nc.dram_tensor("wsB", [H, KB * P], bf16, kind="ExternalInput")
    ws8 = nc.dram_tensor("ws8", [H, KF * P], f8e4, kind="ExternalInput")
    whT = nc.dram_tensor("whT", [H, H], bf16, kind="ExternalInput")
    decT = nc.dram_tensor("decT", [P, HC * NB], bf16, kind="ExternalInput")
    vcol = nc.dram_tensor("vcol", [P, HC], bf16, kind="ExternalInput")
    mneg = nc.dram_tensor("mneg", [NB, P, L // 4], f32, kind="ExternalInput")
    ctx_out = nc.dram_tensor("ctx", [NB, H], f32, kind="ExternalOutput")
    attn_out = nc.dram_tensor("attn", [NB, L], f32, kind="ExternalOutput")

    with tile.TileContext(nc) as tc:
        with (
            tc.tile_pool(name="const", bufs=1) as cpool,
            tc.tile_pool(name="et", bufs=2) as etpool,
            tc.tile_pool(name="et8", bufs=2) as et8pool,
            tc.tile_pool(name="en", bufs=2) as enpool,
            tc.tile_pool(name="work", bufs=3) as wpool,
            tc.tile_pool(name="rows", bufs=2) as rpool,
            tc.tile_pool(name="ps", bufs=2, space=bass.MemorySpace.PSUM) as ppool,
            tc.tile_pool(name="ps1", bufs=1, space=bass.MemorySpace.PSUM) as ppool1,
            tc.tile_pool(name="psrow", bufs=2, space=bass.MemorySpace.PSUM) as prow,
        ):
            # ---- load replicated weights; order shapes DMA queue order:
            # dec+wh first (feeds dh matmuls), then ws / et(n=0)
            # interleaved so the main matmul stream can start early ----
            ws8_sb = cpool.tile([P, HC, KF * P], f8e4)  # [p,hc,k]=ws8[hc*P+p,k]
            wsB_sb = cpool.tile([P, HC, KB * P], bf16)
            wh_sb = cpool.tile([P, HC, H], bf16)
            dec_sb = cpool.tile([P, HC, NB], bf16)
            nc.sync.dma_start(dec_sb[:], decT[:, :])
            for hc in range(HC):
                nc.sync.dma_start(wh_sb[:, hc, :], whT[hc * P:(hc + 1) * P, :])
            v_sb = cpool.tile([P, HC], bf16)
            nc.sync.dma_start(v_sb[:], vcol[:, :])

            et0_sb = et8pool.tile([P, HC, L], f8e4, tag="et8")
            etb0_sb = etpool.tile([P, HC, L], bf16, tag="et")
            for hc in range(HC):
                nc.sync.dma_start(ws8_sb[:, hc, :], ws8[hc * P:(hc + 1) * P, :])
                nc.sync.dma_start(wsB_sb[:, hc, :], wsB[hc * P:(hc + 1) * P, :])
                nc.sync.dma_start(et0_sb[:, hc, :], eT8[0, hc * P:(hc + 1) * P, :])
                nc.sync.dma_start(etb0_sb[:, hc, :], eT[0, hc * P:(hc + 1) * P, :])

            # ---- PE warmup: ~4us of dense dummy matmuls so the HAM
            # clock gate reaches 8/8 before the real stream starts ----
            warm_sb = cpool.tile([P, P], bf16)
            nc.vector.memset(warm_sb[:], 0.0)
            warm_ps = ppool1.tile([P, P], f32, tag="pc")
            for i in range(190):
                nc.tensor.matmul(warm_ps[:], warm_sb[:], warm_sb[:],
                                 start=True, stop=True)

            # ---- dh^T[k, n] = sum_h W_h[k, h] * dec[n, h] ----
            dhT_sb = cpool.tile([P, KC, NB], f32)

            def emit_dh():
                for kc in range(KC):
                    ps = ppool.tile([P, NB], f32, tag="ehps")
                    for hc in range(HC):
                        nc.tensor.matmul(
                            ps[:],
                            wh_sb[:, hc, kc * P:(kc + 1) * P],
                            dec_sb[:, hc, :],
                            start=(hc == 0), stop=(hc == HC - 1))
                    nc.vector.tensor_copy(dhT_sb[:, kc, :], ps[:])

            # ---- fused per-batch pipeline ----
            # ones on every partition (outer-product rhs for any row base)
            ones_sb = cpool.tile([P, 1], bf16)
            nc.vector.memset(ones_sb[:], 1.0)
            # Z-broadcast selector: ones at partitions {0,32,64,96} -> matmul
            # broadcasts the sum of the 4 per-quarter softmax sums to all
            # 128 output partitions
            selbc_sb = cpool.tile([P, P], f32)
            nc.vector.memset(selbc_sb[:], 0.0)
            for j in range(4):
                nc.vector.memset(selbc_sb[32 * j:32 * j + 1, :], 1.0)
            # per-batch mask tiles in split-row layout
            mneg_sb_all = cpool.tile([P, NB, L // 4], f32)
            for n in range(NB):
                nc.sync.dma_start(mneg_sb_all[:, n, :], mneg[n, :, :])
            # scrub the score PSUM slot once: quarters only ever write their
            # 4 rows; stale bits elsewhere must not be NaN/huge (exp reads
            # the full tile)
            sc_init_a = prow.tile([P, L // 4], f32, tag="row")
            nc.vector.memset(sc_init_a[:], 0.0)
            sc_init_b = prow.tile([P, L // 4], f32, tag="row")
            nc.vector.memset(sc_init_b[:], 0.0)
            QL = L // 4  # 256; quarter j lives at psum row 32j, cols 0:QL

            def sc_quads(sc_ps, n, kc, th):
                # score quarters: column group j -> psum row 32j, cols 0:QL
                for j in range(4):
                    nc.tensor.matmul(
                        sc_ps[32 * j:32 * j + 1, :],
                        v_sb[:, kc:kc + 1],
                        th[:, j * QL:(j + 1) * QL],
                        start=(kc == 0), stop=(kc == KC - 1),
                        tile_position=(0, 32 * j))

            def emit_tail(n, sc_ps, en_sb):
                # masked softmax, no max-subtraction (|score| <= sum|v| ~ 26,
                # exp stays in f32 range; mask adds -1e30 pre-exp)
                sc_m = rpool.tile([P, QL], f32, tag="scrow")
                nc.vector.tensor_add(sc_m[:], sc_ps[:], mneg_sb_all[:, n, :])
                prob = rpool.tile([P, QL], f32, tag="prob")
                zs4 = wpool.tile([P, 1], f32, tag="z4")
                nc.scalar.activation(prob[:], sc_m[:], EXP, accum_out=zs4[:])
                z_ps = ppool1.tile([P, 1], f32, tag="pc")
                nc.tensor.matmul(z_ps[:], selbc_sb[:], zs4[:],
                                 start=True, stop=True)
                rzb = wpool.tile([P, 1], f32, tag="rz")
                nc.vector.reciprocal(rzb[:], z_ps[:])
                arow_b = wpool.tile([P, QL], bf16, tag="arowb")
                nc.vector.tensor_scalar_mul(arow_b[:], prob[:], rzb[:])
                arow_f = rpool.tile([P, QL], f32, tag="arowf")
                nc.vector.tensor_scalar_mul(arow_f[:], prob[:], rzb[:])
                for j in range(4):
                    nc.sync.dma_start(
                        attn_out[n:n + 1, j * QL:(j + 1) * QL],
                        arow_f[32 * j:32 * j + 1, :])

                # transpose attn quarters -> columns via outer products
                ac_ps = ppool1.tile([P, LC], f32, tag="pc")
                for lc in range(LC):
                    j = lc // 2
                    nc.tensor.matmul(ac_ps[:, lc:lc + 1],
                                     arow_b[32 * j:32 * j + 1,
                                            (lc % 2) * P:(lc % 2 + 1) * P],
                                     ones_sb[32 * j:32 * j + 1, :],
                                     start=True, stop=True,
                                     tile_position=(32 * j, 0))
                acol = wpool.tile([P, LC], bf16, tag="acol")
                nc.vector.tensor_copy(acol[:], ac_ps[:])

                # context[n, h] = sum_l attn[l] E[l, h]; 4 column groups
                # compute disjoint h-quarters at psum rows 32j, cols 0:QH
                QH = H // 4
                cx_ps = ppool1.tile([P, QH], f32, tag="pc")
                for lc in range(LC):
                    for j in range(4):
                        nc.tensor.matmul(
                            cx_ps[32 * j:32 * j + 1, :],
                            acol[:, lc:lc + 1],
                            en_sb[:, lc, j * QH:(j + 1) * QH],
                            start=(lc == 0), stop=(lc == LC - 1),
                            tile_position=(0, 32 * j))
                cx_row = rpool.tile([P, QH], f32, tag="cxrow")
                nc.vector.tensor_copy(cx_row[:], cx_ps[:])
                for j in range(4):
                    nc.sync.dma_start(ctx_out[n:n + 1, j * QH:(j + 1) * QH],
                                      cx_row[32 * j:32 * j + 1, :])

            pend = None
            for n in range(NB):
                if n == 0:
                    et8_sb = et0_sb
                    et_sb = etb0_sb
                else:
                    et8_sb = et8pool.tile([P, HC, L], f8e4, tag="et8")
                    et_sb = etpool.tile([P, HC, L], bf16, tag="et")
                    for hc in range(HC):
                        nc.sync.dma_start(et8_sb[:, hc, :],
                                          eT8[n, hc * P:(hc + 1) * P, :])
                        nc.sync.dma_start(et_sb[:, hc, :],
                                          eT[n, hc * P:(hc + 1) * P, :])
                en_sb = enpool.tile([P, LC, H], bf16, tag="en")
                for lc in range(LC):
                    nc.sync.dma_start(en_sb[:, lc, :],
                                      eN[n, lc * P:(lc + 1) * P, :])

                sc_ps = prow.tile([P, QL], f32, tag="row")
                prev_th = None
                for kc in range(KC):
                    eh_ps = ppool.tile([P, L], f32, tag="ehps")
                    if kc < KF:
                        # fp8 DoubleRow: one matmul covers two h-chunks
                        for hcp in range(HC // 2):
                            for lt in range(2):
                                nc.tensor.matmul(
                                    eh_ps[:, lt * 512:(lt + 1) * 512],
                                    ws8_sb[:, 2 * hcp:2 * hcp + 2,
                                           kc * P:(kc + 1) * P],
                                    et8_sb[:, 2 * hcp:2 * hcp + 2,
                                           lt * 512:(lt + 1) * 512],
                                    start=(hcp == 0), stop=(hcp == HC // 2 - 1),
                                    perf_mode=DROW)
                    else:
                        for hc in range(HC):
                            for lt in range(2):
                                nc.tensor.matmul(
                                    eh_ps[:, lt * 512:(lt + 1) * 512],
                                    wsB_sb[:, hc, (kc - KF) * P:(kc - KF + 1) * P],
                                    et_sb[:, hc, lt * 512:(lt + 1) * 512],
                                    start=(hc == 0), stop=(hc == HC - 1))
                    if n == 0 and kc == 0:
                        emit_dh()
                    th = wpool.tile([P, L], bf16, tag="tanh")
                    nc.scalar.activation(th[:], eh_ps[:], TANH,
                                         bias=dhT_sb[:, kc, n:n + 1],
                                         scale=(1.0 / WS_SCALE if kc < KF
                                                else 1.0))
                    if kc == 2 and pend is not None:
                        # previous batch's softmax/attn/context, emitted two
                        # eh groups into this batch so the PE never waits
                        emit_tail(*pend)
                        pend = None
                    if prev_th is not None:
                        sc_quads(sc_ps, n, kc - 1, prev_th)
                    prev_th = th
                sc_quads(sc_ps, n, KC - 1, prev_th)
                pend = (n, sc_ps, en_sb)
            emit_tail(*pend)

    nc.compile()
    return nc


def kernel(decoder_hidden, encoder_hiddens, mask, W_h, W_s, v):
    global last_exec_time_ns, last_trace
    from concourse.bass_utils import run_bass_kernel_spmd

    bf16 = ml_dtypes.bfloat16
    f8 = ml_dtypes.float8_e4m3
    dec = np.asarray(decoder_hidden, np.float32)
    enc = np.asarray(encoder_hiddens, np.float32)
    msk = np.asarray(mask)
    W_h = np.asarray(W_h, np.float32)
    W_s = np.asarray(W_s, np.float32)
    v = np.asarray(v, np.float32)

    # permute the k dimension so |v_k| is ascending: low-|v| chunks carry
    # little score-error sensitivity and run in fp8
    pi = np.argsort(np.abs(v), kind="stable")
    W_s = W_s[pi]
    W_h = W_h[pi]
    v = v[pi]

    wsT = np.ascontiguousarray(W_s.T)                      # [h, k] permuted
    ws8 = np.ascontiguousarray(wsT[:, :KF * P] * WS_SCALE).astype(f8)
    wsB = np.ascontiguousarray(wsT[:, KF * P:]).astype(bf16)
    whT = np.ascontiguousarray(W_h.T).astype(bf16)
    vcol = np.ascontiguousarray(v.reshape(HC, P).T).astype(bf16)
    NEG = np.float32(-1e30)
    mneg_rows = np.where(msk, NEG, np.float32(0.0)).astype(np.float32)  # [N, L]
    QL = L // 4
    mneg4 = np.full((N, P, QL), NEG, np.float32)
    for j in range(4):
        mneg4[:, 32 * j, :] = mneg_rows[:, j * QL:(j + 1) * QL]

    enc_b = enc.astype(bf16)

    in_maps = []
    for c in range(N_CORES):
        s = slice(c * NB, (c + 1) * NB)
        encT = enc[s].transpose(0, 2, 1)
        in_maps.append({
            "eT": np.ascontiguousarray(encT.astype(bf16)),
            "eT8": np.ascontiguousarray(encT.astype(f8)),
            "eN": np.ascontiguousarray(enc_b[s]),
            "wsB": wsB,
            "ws8": ws8,
            "whT": whT,
            "decT": np.ascontiguousarray(
                dec[s].T.reshape(HC, P, NB).transpose(1, 0, 2).reshape(P, HC * NB)
            ).astype(bf16),
            "vcol": vcol,
            "mneg": np.ascontiguousarray(mneg4[s]),
        })

    if "nc" not in _cache:
        _cache["nc"] = _build()
    nc = _cache["nc"]

    trace = bool(int(os.environ.get("BASS_KERNEL_TRACE", "0")))
    res = run_bass_kernel_spmd(nc, in_maps, core_ids=list(range(N_CORES)),
                               trace=trace)
    last_exec_time_ns = res.exec_time_ns
    last_trace = res.instructions_and_trace

    context = np.concatenate([res.results[c]["ctx"] for c in range(N_CORES)], 0)
    attn_w = np.concatenate([res.results[c]["attn"] for c in range(N_CORES)], 0)
    return (context.astype(np.float32), attn_w.astype(np.float32))
